# revision 1
# baseline (speedup 1.0000x reference)
"""Bass/Trainium2 kernel for nn_GAT_25082609009415.

GAT: g = x[46,131072] @ W1[131072,2048] -> 8-head masked attention ->
ELU -> h @ W2[2048,64] -> 1-head attention -> mean -> MLP(46->12->1) -> sigmoid.

Strategy (8 NeuronCores): shard the contraction (K) dim of the dominant
GEMM: core c streams W1[16384c:16384(c+1), :] (134 MB — the memory-bound
roofline) and x[:, 16384c:16384(c+1)], accumulates a partial g[46,2048]
in PSUM, AllReduce(add) over the 8 cores, then every core runs the tiny
attention/MLP tail redundantly; core 0's scalar output is returned.
"""
import numpy as np

import concourse.bass as bass
import concourse.bacc as bacc
import concourse.tile as tile
from concourse import mybir
from concourse.bass_utils import run_bass_kernel_spmd

N = 46
KTOT = 131072
HID = 2048
HEADS = 8
F1 = HID // HEADS          # 256 features / head
OUTF = 64
NCORES = 8
KC = KTOT // NCORES        # 16384 contraction elems per core
KT = KC // 128             # 128 k-tiles per core
KT2 = HID // 128           # 16 k-tiles for layer-2 GEMM / gT
MASK_NEG = -1.0e4          # exp(<= -9900) == 0.0f exactly; matches where(adj,e,-1e30)

F32 = mybir.dt.float32
F32R = mybir.dt.float32r
AX = mybir.AxisListType
OP = mybir.AluOpType
ACTF = mybir.ActivationFunctionType

# The BIR verifier requires every producer feeding an FP32r matmul to be
# typed float32r, so the x/W1 dataflow (DRAM tensor -> SBUF tile ->
# transpose psum) is declared float32r end-to-end. Set False for exact fp32.
USE_F32R = True
GEMM_DT = F32R if USE_F32R else F32


def build():
    nc = bacc.Bacc(
        "TRN2",
        target_bir_lowering=False,
        debug=False,
        enable_asserts=False,
        num_devices=NCORES,
    )
    xs = nc.dram_tensor("xs", [N, KC], GEMM_DT, kind="ExternalInput")
    w1 = nc.dram_tensor("w1", [128, KT * HID], GEMM_DT, kind="ExternalInput")
    w2r = nc.dram_tensor("w2r", [128, KT2 * OUTF], F32, kind="ExternalInput")
    adjb = nc.dram_tensor("adjb", [N, N], F32, kind="ExternalInput")
    asrc = nc.dram_tensor("asrc", [128, KT2], F32, kind="ExternalInput")
    adst = nc.dram_tensor("adst", [128, KT2], F32, kind="ExternalInput")
    a2s = nc.dram_tensor("a2s", [OUTF, 1], F32, kind="ExternalInput")
    a2d = nc.dram_tensor("a2d", [OUTF, 1], F32, kind="ExternalInput")
    mw1 = nc.dram_tensor("mw1", [N, 12], F32, kind="ExternalInput")
    mb1 = nc.dram_tensor("mb1", [1, 12], F32, kind="ExternalInput")
    mw2t = nc.dram_tensor("mw2t", [1, 12], F32, kind="ExternalInput")
    mb2 = nc.dram_tensor("mb2", [1, 1], F32, kind="ExternalInput")
    ident = nc.dram_tensor("ident", [128, 128], F32, kind="ExternalInput")
    identr = nc.dram_tensor("identr", [128, 128], F32R, kind="ExternalInput")
    out = nc.dram_tensor("out", [1, 1], F32, kind="ExternalOutput")

    with tile.TileContext(nc) as tc:
        with (
            tc.tile_pool(name="psT", bufs=2, space="PSUM") as psT,
            tc.tile_pool(name="const", bufs=1) as cst,
            tc.tile_pool(name="sbx", bufs=2) as sbx,
            tc.tile_pool(name="sbxT", bufs=1) as sbxT,
            tc.tile_pool(name="sbw1", bufs=3) as sbw1,
            tc.tile_pool(name="sbbig", bufs=1) as sbbig,
            tc.tile_pool(name="sbmed", bufs=1) as sbmed,
            tc.tile_pool(name="sbsm", bufs=1) as sbsm,
            tc.tile_pool(name="dram", bufs=1, space="DRAM") as dram,
        ):
            # ---- constants ----
            ident_sb = cst.tile([128, 128], F32, tag="ident")
            nc.sync.dma_start(ident_sb[:], ident.ap())
            identr_sb = cst.tile([128, 128], F32R, tag="identr")
            nc.sync.dma_start(identr_sb[:], identr.ap())
            adjb_sb = cst.tile([N, N], F32, tag="adjb")
            nc.sync.dma_start(adjb_sb[:], adjb.ap())
            asrc_sb = cst.tile([128, KT2], F32, tag="asrc")
            nc.sync.dma_start(asrc_sb[:], asrc.ap())
            adst_sb = cst.tile([128, KT2], F32, tag="adst")
            nc.sync.dma_start(adst_sb[:], adst.ap())
            w2_sb = cst.tile([128, KT2 * OUTF], F32, tag="w2")
            nc.sync.dma_start(w2_sb[:], w2r.ap())
            a2s_sb = cst.tile([OUTF, 1], F32, tag="a2s")
            nc.sync.dma_start(a2s_sb[:], a2s.ap())
            a2d_sb = cst.tile([OUTF, 1], F32, tag="a2d")
            nc.sync.dma_start(a2d_sb[:], a2d.ap())
            mw1_sb = cst.tile([N, 12], F32, tag="mw1")
            nc.sync.dma_start(mw1_sb[:], mw1.ap())
            mb1_sb = cst.tile([1, 12], F32, tag="mb1")
            nc.sync.dma_start(mb1_sb[:], mb1.ap())
            mw2t_sb = cst.tile([1, 12], F32, tag="mw2t")
            nc.sync.dma_start(mw2t_sb[:], mw2t.ap())
            mb2_sb = cst.tile([1, 1], F32, tag="mb2")
            nc.sync.dma_start(mb2_sb[:], mb2.ap())

            # ---- phase A: load x slice, transpose to xT tiles [128, 46] ----
            xT_all = sbxT.tile([128, KT, N], GEMM_DT, tag="xT")
            XCH = 2048                      # x chunk width
            for cch in range(KC // XCH):
                xc_sb = sbx.tile([N, XCH], GEMM_DT, tag="xc")
                nc.sync.dma_start(xc_sb[:], xs.ap()[:, XCH * cch:XCH * (cch + 1)])
                for j in range(XCH // 128):
                    k = cch * (XCH // 128) + j
                    pt = psT.tile([128, N], GEMM_DT, tag="tp")
                    nc.tensor.transpose(
                        pt[:],
                        xc_sb[:, 128 * j:128 * (j + 1)],
                        identr_sb[:N, :N] if USE_F32R else ident_sb[:N, :N],
                    )
                    nc.vector.tensor_copy(xT_all[:, k, :], pt[:])

            # ---- phase B: main GEMM  g_partial = x_c @ W1_c  ----
            with tc.tile_pool(name="psA", bufs=1, space="PSUM") as psA:
                g_ps = psA.tile([N, HID], F32, tag="g")
                TPD = 2                       # k-tiles per DMA
                for k2 in range(KT // TPD):
                    w1_sb = sbw1.tile([128, TPD * HID], GEMM_DT, tag="w1")
                    nc.sync.dma_start(
                        w1_sb[:],
                        w1.ap()[:, TPD * HID * k2:TPD * HID * (k2 + 1)],
                    )
                    for t in range(TPD):
                        k = TPD * k2 + t
                        lhs = xT_all[:, k, :]
                        for nn in range(HID // 512):
                            nc.tensor.matmul(
                                g_ps[:, 512 * nn:512 * (nn + 1)],
                                lhs,
                                w1_sb[:, HID * t + 512 * nn:HID * t + 512 * (nn + 1)],
                                start=(k == 0),
                                stop=(k == KT - 1),
                            )
                gp_sb = sbbig.tile([N, HID], F32, tag="gp")
                for nn in range(HID // 512):
                    nc.vector.tensor_copy(
                        gp_sb[:, 512 * nn:512 * (nn + 1)],
                        g_ps[:, 512 * nn:512 * (nn + 1)],
                    )

            # ---- phase C: AllReduce partial g over the 8 cores ----
            cc_in = dram.tile([N, HID], F32, tag="ccin")
            cc_out = dram.tile([N, HID], F32, tag="ccout")
            nc.sync.dma_start(cc_in[:], gp_sb[:])
            nc.gpsimd.collective_compute(
                "AllReduce",
                OP.add,
                replica_groups=[list(range(NCORES))],
                ins=[cc_in[:].opt()],
                outs=[cc_out[:].opt()],
            )
            g_sb = sbbig.tile([N, HID], F32, tag="g")
            nc.sync.dma_start(g_sb[:], cc_out[:])

            with (
                tc.tile_pool(name="psH", bufs=1, space="PSUM") as psH,
                tc.tile_pool(name="psS", bufs=1, space="PSUM") as psS,
            ):
                # ---- phase D: attention layer 1 (8 heads, f=256) ----
                gT_all = sbmed.tile([128, KT2, N], F32, tag="gT")
                for k in range(KT2):
                    pt = psT.tile([128, N], F32, tag="tp")
                    nc.tensor.transpose(
                        pt[:], g_sb[:, 128 * k:128 * (k + 1)], ident_sb[:N, :N]
                    )
                    nc.vector.tensor_copy(gT_all[:, k, :], pt[:])

                # e_src[i,h] / e_dst row [1, (h,j)] via PE
                esrc_ps = psS.tile([N, HEADS], F32, tag="ev")
                for k in range(KT2):
                    h = k // 2
                    nc.tensor.matmul(
                        esrc_ps[:, h:h + 1],
                        gT_all[:, k, :],
                        asrc_sb[:, k:k + 1],
                        start=(k % 2 == 0),
                        stop=(k % 2 == 1),
                    )
                esrc_sb = sbsm.tile([N, HEADS], F32, tag="esrc")
                nc.vector.tensor_copy(esrc_sb[:], esrc_ps[:])

                edst_ps = psS.tile([1, HEADS * N], F32, tag="er")
                for k in range(KT2):
                    h = k // 2
                    nc.tensor.matmul(
                        edst_ps[0:1, N * h:N * (h + 1)],
                        adst_sb[:, k:k + 1],
                        gT_all[:, k, :],
                        start=(k % 2 == 0),
                        stop=(k % 2 == 1),
                    )
                edst_sb = sbsm.tile([1, HEADS * N], F32, tag="edst")
                nc.vector.tensor_copy(edst_sb[:], edst_ps[:])
                ebc_sb = sbmed.tile([N, HEADS * N], F32, tag="ebc")
                nc.gpsimd.partition_broadcast(ebc_sb[:], edst_sb[:])

                # e = leaky_relu(e_src + e_dst, 0.2) + adj_bias ; u = exp(e)
                e_sb = sbmed.tile([N, HEADS, N], F32, tag="e")
                nc.vector.tensor_add(
                    e_sb[:],
                    ebc_sb[:].rearrange("p (h j) -> p h j", h=HEADS),
                    esrc_sb[:].unsqueeze(2).broadcast_to([N, HEADS, N]),
                )
                t02 = sbmed.tile([N, HEADS, N], F32, tag="t02")
                nc.vector.tensor_scalar_mul(t02[:], e_sb[:], 0.2)
                nc.vector.tensor_max(e_sb[:], e_sb[:], t02[:])
                nc.vector.tensor_add(
                    e_sb[:],
                    e_sb[:],
                    adjb_sb[:].unsqueeze(1).broadcast_to([N, HEADS, N]),
                )
                u_sb = sbmed.tile([N, HEADS, N], F32, tag="u")
                nc.scalar.activation(u_sb[:], e_sb[:], ACTF.Exp)
                s_sb = sbsm.tile([N, HEADS], F32, tag="s")
                nc.vector.tensor_reduce(s_sb[:], u_sb[:], axis=AX.X, op=OP.add)
                r_sb = sbsm.tile([N, HEADS], F32, tag="r")
                nc.vector.reciprocal(r_sb[:], s_sb[:])

                # h1[:, h] = (u_h @ g_h) * r_h   (transpose u_h, PE matmul, scale)
                h1_ps = psH.tile([N, HID], F32, tag="big")
                for h in range(HEADS):
                    ut_ps = psT.tile([N, N], F32, tag="tp")
                    nc.tensor.transpose(ut_ps[:], u_sb[:, h, :], ident_sb[:N, :N])
                    ut_sb = sbsm.tile([N, N], F32, tag="ut")
                    nc.vector.tensor_copy(ut_sb[:], ut_ps[:])
                    nc.tensor.matmul(
                        h1_ps[:, F1 * h:F1 * (h + 1)],
                        ut_sb[:],
                        g_sb[:, F1 * h:F1 * (h + 1)],
                        start=True,
                        stop=True,
                    )
                h1_sb = sbbig.tile([N, HID], F32, tag="h1")
                for h in range(HEADS):
                    nc.vector.tensor_scalar(
                        h1_sb[:, F1 * h:F1 * (h + 1)],
                        h1_ps[:, F1 * h:F1 * (h + 1)],
                        r_sb[:, h:h + 1],
                        None,
                        OP.mult,
                    )

                # ELU:  h = max(h1,0) + exp(min(h1,0)) - 1
                tneg = sbbig.tile([N, HID], F32, tag="tneg")
                nc.vector.tensor_scalar_min(tneg[:], h1_sb[:], 0.0)
                texp = sbbig.tile([N, HID], F32, tag="texp")
                nc.scalar.activation(texp[:], tneg[:], ACTF.Exp)
                nc.vector.tensor_scalar_max(h1_sb[:], h1_sb[:], 0.0)
                h_sb = sbbig.tile([N, HID], F32, tag="h")
                nc.vector.scalar_tensor_tensor(
                    h_sb[:], texp[:], -1.0, h1_sb[:], op0=OP.add, op1=OP.add
                )

                # ---- phase E: layer 2 GEMM + 1-head attention + MLP ----
                hT_all = sbmed.tile([128, KT2, N], F32, tag="hT")
                for k in range(KT2):
                    pt = psT.tile([128, N], F32, tag="tp")
                    nc.tensor.transpose(
                        pt[:], h_sb[:, 128 * k:128 * (k + 1)], ident_sb[:N, :N]
                    )
                    nc.vector.tensor_copy(hT_all[:, k, :], pt[:])
                g2_ps = psH.tile([N, OUTF], F32, tag="big")
                for k in range(KT2):
                    nc.tensor.matmul(
                        g2_ps[:],
                        hT_all[:, k, :],
                        w2_sb[:, OUTF * k:OUTF * (k + 1)],
                        start=(k == 0),
                        stop=(k == KT2 - 1),
                    )
                g2_sb = sbsm.tile([N, OUTF], F32, tag="g2")
                nc.vector.tensor_copy(g2_sb[:], g2_ps[:])

                g2T_ps = psT.tile([OUTF, N], F32, tag="tp")
                nc.tensor.transpose(g2T_ps[:], g2_sb[:], ident_sb[:N, :N])
                g2T_sb = sbsm.tile([OUTF, N], F32, tag="g2T")
                nc.vector.tensor_copy(g2T_sb[:], g2T_ps[:])

                e2s_ps = psS.tile([N, 1], F32, tag="ev")
                nc.tensor.matmul(e2s_ps[:], g2T_sb[:], a2s_sb[:], start=True, stop=True)
                e2s_sb = sbsm.tile([N, 1], F32, tag="e2s")
                nc.vector.tensor_copy(e2s_sb[:], e2s_ps[:])
                e2d_ps = psS.tile([1, N], F32, tag="er")
                nc.tensor.matmul(e2d_ps[:], a2d_sb[:], g2T_sb[:], start=True, stop=True)
                e2d_sb = sbsm.tile([1, N], F32, tag="e2d")
                nc.vector.tensor_copy(e2d_sb[:], e2d_ps[:])
                e2bc_sb = sbsm.tile([N, N], F32, tag="e2bc")
                nc.gpsimd.partition_broadcast(e2bc_sb[:], e2d_sb[:])

                e2_sb = sbsm.tile([N, N], F32, tag="e2")
                nc.vector.tensor_add(
                    e2_sb[:], e2bc_sb[:], e2s_sb[:].broadcast_to([N, N])
                )
                t22 = sbsm.tile([N, N], F32, tag="t22")
                nc.vector.tensor_scalar_mul(t22[:], e2_sb[:], 0.2)
                nc.vector.tensor_max(e2_sb[:], e2_sb[:], t22[:])
                nc.vector.tensor_add(e2_sb[:], e2_sb[:], adjb_sb[:])
                u2_sb = sbsm.tile([N, N], F32, tag="u2")
                nc.scalar.activation(u2_sb[:], e2_sb[:], ACTF.Exp)
                s2_sb = sbsm.tile([N, 1], F32, tag="s2")
                nc.vector.tensor_reduce(s2_sb[:], u2_sb[:], axis=AX.X, op=OP.add)
                r2_sb = sbsm.tile([N, 1], F32, tag="r2")
                nc.vector.reciprocal(r2_sb[:], s2_sb[:])

                u2T_ps = psT.tile([N, N], F32, tag="tp")
                nc.tensor.transpose(u2T_ps[:], u2_sb[:], ident_sb[:N, :N])
                u2T_sb = sbsm.tile([N, N], F32, tag="u2T")
                nc.vector.tensor_copy(u2T_sb[:], u2T_ps[:])
                o2_ps = psH.tile([N, OUTF], F32, tag="big")
                nc.tensor.matmul(o2_ps[:], u2T_sb[:], g2_sb[:], start=True, stop=True)
                o2_sb = sbsm.tile([N, OUTF], F32, tag="o2")
                nc.vector.tensor_scalar(
                    o2_sb[:], o2_ps[:], r2_sb[:, 0:1], None, OP.mult
                )
                # mean over the 64 features folded into host-prescaled mw1 (/64)
                m_sb = sbsm.tile([N, 1], F32, tag="m")
                nc.vector.tensor_reduce(m_sb[:], o2_sb[:], axis=AX.X, op=OP.add)

                z1_ps = psS.tile([1, 12], F32, tag="er")
                nc.tensor.matmul(z1_ps[:], m_sb[:], mw1_sb[:], start=True, stop=True)
                z1_sb = sbsm.tile([1, 12], F32, tag="z1")
                nc.vector.tensor_add(z1_sb[:], z1_ps[:], mb1_sb[:])
                zt_sb = sbsm.tile([1, 12], F32, tag="zt")
                nc.vector.tensor_mul(zt_sb[:], z1_sb[:], mw2t_sb[:])
                z2_sb = sbsm.tile([1, 1], F32, tag="z2")
                nc.vector.tensor_reduce(z2_sb[:], zt_sb[:], axis=AX.X, op=OP.add)
                res_sb = sbsm.tile([1, 1], F32, tag="res")
                nc.scalar.activation(
                    res_sb[:], z2_sb[:], ACTF.Sigmoid, bias=mb2_sb[:, 0:1]
                )
                nc.sync.dma_start(out.ap(), res_sb[:])

    nc.compile()
    return nc


_NC_CACHE = []


def _get_nc():
    if not _NC_CACHE:
        _NC_CACHE.append(build())
    return _NC_CACHE[0]


def _prep_in_maps(x, adj, W1, a1, W2, a2, mw1, mb1, mw2, mb2):
    adjb = np.where(adj[:, :, 0], np.float32(0.0), np.float32(MASK_NEG)).astype(
        np.float32
    )
    # a1 [8, 512]: src half / dst half, flattened h-major to match g columns,
    # then laid out [128 partitions, 16 k-tiles]
    asrc = np.ascontiguousarray(
        a1[:, :F1].reshape(KT2, 128).T
    )
    adst = np.ascontiguousarray(a1[:, F1:].reshape(KT2, 128).T)
    w2r = np.ascontiguousarray(
        W2.reshape(KT2, 128, OUTF).transpose(1, 0, 2).reshape(128, KT2 * OUTF)
    )
    a2sv = np.ascontiguousarray(a2[0, :OUTF].reshape(OUTF, 1))
    a2dv = np.ascontiguousarray(a2[0, OUTF:].reshape(OUTF, 1))
    shared = {
        "adjb": adjb,
        "asrc": asrc,
        "adst": adst,
        "w2r": w2r,
        "a2s": a2sv,
        "a2d": a2dv,
        "mw1": np.ascontiguousarray(mw1 / np.float32(OUTF)),
        "mb1": mb1.reshape(1, 12).astype(np.float32),
        "mw2t": np.ascontiguousarray(mw2.reshape(1, 12)),
        "mb2": mb2.reshape(1, 1).astype(np.float32),
        "ident": np.eye(128, dtype=np.float32),
        "identr": np.eye(128, dtype=np.float32),
    }
    in_maps = []
    for c in range(NCORES):
        m = dict(shared)
        m["xs"] = np.ascontiguousarray(x[:, KC * c:KC * (c + 1)])
        w1c = W1[KC * c:KC * (c + 1), :].reshape(KT, 128, HID)
        m["w1"] = np.ascontiguousarray(
            w1c.transpose(1, 0, 2).reshape(128, KT * HID)
        )
        in_maps.append(m)
    return in_maps


def kernel(**inputs):
    x = np.asarray(inputs["x"], dtype=np.float32)
    adj = np.asarray(inputs["adj_mat"]).astype(bool).reshape(N, N, 1)
    W1 = np.asarray(inputs["W1"], dtype=np.float32)
    a1 = np.asarray(inputs["a1"], dtype=np.float32)
    W2 = np.asarray(inputs["W2"], dtype=np.float32)
    a2 = np.asarray(inputs["a2"], dtype=np.float32)
    mw1 = np.asarray(inputs["mlp_w1"], dtype=np.float32)
    mb1 = np.asarray(inputs["mlp_b1"], dtype=np.float32)
    mw2 = np.asarray(inputs["mlp_w2"], dtype=np.float32)
    mb2 = np.asarray(inputs["mlp_b2"], dtype=np.float32)

    nc = _get_nc()
    in_maps = _prep_in_maps(x, adj, W1, a1, W2, a2, mw1, mb1, mw2, mb2)
    res = run_bass_kernel_spmd(nc, in_maps, core_ids=list(range(NCORES)))
    return res.results[0]["out"].reshape(1).astype(np.float32)



# revision 3
# speedup vs baseline: 3.4865x; 3.4865x over previous
"""Bass/Trainium2 kernel for nn_GAT_25082609009415.

GAT: g = x[46,131072] @ W1[131072,2048] -> 8-head masked attention ->
ELU -> h @ W2[2048,64] -> 1-head attention -> mean -> MLP(46->12->1) -> sigmoid.

Strategy (8 NeuronCores), v2:
  * K-shard the dominant GEMM: core c streams W1[16384c:16384(c+1), :]
    in fp8e4 (host-scaled by 2^12; x scaled by 2^5) -- 33.5 MB/core,
    4x fewer HBM bytes than the fp32 baseline.  x arrives
    host-pretransposed as [128, kt, 46] fp8, so no on-device transposes.
  * W1 is laid out in G=4 column groups of 512; each group's partial
    g accumulates in its own PSUM bank and is AllReduced while the next
    group's GEMM streams -- 3 of 4 collectives are hidden.
  * e_src/e_dst are linear in g, so their per-core partials are computed
    from the local partial g and ride along in the last AllReduce.
  * Tail attention avoids gT entirely: e_src/e_dst via vector
    mult+reduce, e_dst broadcast via tiny PE outer-product matmuls,
    h1 computed directly transposed (h1T) so layer-2 needs no
    transposes.  Single activation table (Exp), preloaded during the
    GEMM; sigmoid is computed via exp+reciprocal.
"""
import numpy as np
import ml_dtypes

import concourse.bass as bass
import concourse.bacc as bacc
import concourse.tile as tile
from concourse import mybir
from concourse.bass_utils import run_bass_kernel_spmd

N = 46
KTOT = 131072
HID = 2048
HEADS = 8
F1 = HID // HEADS          # 256 features / head
OUTF = 64
NCORES = 8
KC = KTOT // NCORES        # 16384 contraction elems per core
KT = KC // 128             # 128 k-tiles per core
KT2 = HID // 128           # 16 k-tiles for layer-2 GEMM
G = 4                      # column groups for pipelined AllReduce
GW = HID // G              # 512 columns per group
HPG = HEADS // G           # heads per group
TPD = 32                   # k-tiles per W1 DMA chunk (16KB/partition)
NCH = KT // TPD            # chunks per group

W_SCALE = float(2 ** 12)   # keeps W1 (~+-0.0028) in fp8e4 normal range
X_SCALE = float(2 ** 5)    # keeps x (~N(0,1)) well under fp8e4 max 240
DESCALE = 1.0 / (W_SCALE * X_SCALE)

F32 = mybir.dt.float32
F16 = mybir.dt.float16
F8 = mybir.dt.float8e4
AX = mybir.AxisListType
OP = mybir.AluOpType
ACTF = mybir.ActivationFunctionType

NP_F8 = ml_dtypes.float8_e4m3


def build():
    nc = bacc.Bacc(
        "TRN2",
        target_bir_lowering=False,
        debug=False,
        enable_asserts=False,
        num_devices=NCORES,
    )
    xs = nc.dram_tensor("xs", [128, KT * N], F8, kind="ExternalInput")
    w1 = nc.dram_tensor("w1", [128, G * KT * GW], F8, kind="ExternalInput")
    w2r = nc.dram_tensor("w2r", [128, KT2 * OUTF], F16, kind="ExternalInput")
    adj01 = nc.dram_tensor("adj01", [N, N], F32, kind="ExternalInput")
    asrcf = nc.dram_tensor("asrcf", [1, HID], F16, kind="ExternalInput")
    adstf = nc.dram_tensor("adstf", [1, HID], F16, kind="ExternalInput")
    a2sf = nc.dram_tensor("a2sf", [1, OUTF], F16, kind="ExternalInput")
    a2df = nc.dram_tensor("a2df", [1, OUTF], F16, kind="ExternalInput")
    sel = nc.dram_tensor("sel", [HEADS, HEADS * N], F32, kind="ExternalInput")
    ones46 = nc.dram_tensor("ones46", [1, N], F32, kind="ExternalInput")
    mw1 = nc.dram_tensor("mw1", [N, 12], F32, kind="ExternalInput")
    mb1 = nc.dram_tensor("mb1", [1, 12], F32, kind="ExternalInput")
    mw2t = nc.dram_tensor("mw2t", [1, 12], F32, kind="ExternalInput")
    mb2 = nc.dram_tensor("mb2", [1, 1], F32, kind="ExternalInput")
    ident = nc.dram_tensor("ident", [N, N], F32, kind="ExternalInput")
    out = nc.dram_tensor("out", [1, 1], F32, kind="ExternalOutput")

    # width of the last collective: g columns + esrc[8] + edst[8]
    W3 = GW + 2 * HEADS

    with tile.TileContext(nc) as tc:
        with (
            tc.tile_pool(name="psT", bufs=2, space="PSUM") as psT,
            tc.tile_pool(name="const", bufs=1) as cst,
            tc.tile_pool(name="sbxT", bufs=1) as sbxT,
            tc.tile_pool(name="sbw1", bufs=5) as sbw1,
            tc.tile_pool(name="sbst", bufs=2) as sbst,
            tc.tile_pool(name="sbg", bufs=1) as sbg,
            tc.tile_pool(name="sbgf", bufs=2) as sbgf,
            tc.tile_pool(name="sbt", bufs=1) as sbt,
            tc.tile_pool(name="sbsm", bufs=1) as sbsm,
            tc.tile_pool(name="dram", bufs=1, space="DRAM") as dram,
        ):
            # ---- phase A: x slices (pretransposed on host) + constants ----
            xT = sbxT.tile([128, KT, N], F8, tag="xT")
            XCH = KT // 4
            for c in range(4):
                q = nc.sync if c % 2 == 0 else nc.scalar
                q.dma_start(
                    xT[:, XCH * c:XCH * (c + 1), :],
                    xs.ap()[:, XCH * N * c:XCH * N * (c + 1)],
                )

            ident_sb = cst.tile([N, N], F32, tag="ident")
            nc.sync.dma_start(ident_sb[:], ident.ap())
            adj01_sb = cst.tile([N, N], F32, tag="adj01")
            nc.sync.dma_start(adj01_sb[:], adj01.ap())
            sel_sb = cst.tile([HEADS, HEADS * N], F32, tag="sel")
            nc.sync.dma_start(sel_sb[:], sel.ap())
            ones46_sb = cst.tile([1, N], F32, tag="ones46")
            nc.sync.dma_start(ones46_sb[:], ones46.ap())
            w2_sb = cst.tile([128, KT2, OUTF], F16, tag="w2")
            nc.sync.dma_start(w2_sb[:], w2r.ap())
            asrc1 = cst.tile([1, HID], F16, tag="asrc1")
            nc.sync.dma_start(asrc1[:], asrcf.ap())
            adst1 = cst.tile([1, HID], F16, tag="adst1")
            nc.sync.dma_start(adst1[:], adstf.ap())
            a2s1 = cst.tile([1, OUTF], F16, tag="a2s1")
            nc.sync.dma_start(a2s1[:], a2sf.ap())
            a2d1 = cst.tile([1, OUTF], F16, tag="a2d1")
            nc.sync.dma_start(a2d1[:], a2df.ap())
            mw1_sb = cst.tile([N, 12], F32, tag="mw1")
            nc.sync.dma_start(mw1_sb[:], mw1.ap())
            mb1_sb = cst.tile([1, 12], F32, tag="mb1")
            nc.sync.dma_start(mb1_sb[:], mb1.ap())
            mw2t_sb = cst.tile([1, 12], F32, tag="mw2t")
            nc.sync.dma_start(mw2t_sb[:], mw2t.ap())
            mb2_sb = cst.tile([1, 1], F32, tag="mb2")
            nc.sync.dma_start(mb2_sb[:], mb2.ap())

            # broadcast attention vectors across the 46 node partitions
            asrcb = cst.tile([N, HID], F16, tag="asrcb")
            nc.gpsimd.partition_broadcast(asrcb[:], asrc1[:])
            adstb = cst.tile([N, HID], F16, tag="adstb")
            nc.gpsimd.partition_broadcast(adstb[:], adst1[:])
            a2sb = cst.tile([N, OUTF], F16, tag="a2sb")
            nc.gpsimd.partition_broadcast(a2sb[:], a2s1[:])
            a2db = cst.tile([N, OUTF], F16, tag="a2db")
            nc.gpsimd.partition_broadcast(a2db[:], a2d1[:])

            # preload the Exp activation table off the critical path
            dummy = sbsm.tile([1, 1], F32, tag="dummy")
            nc.scalar.activation(dummy[:], mb2_sb[:], ACTF.Exp)

            # per-core partial e_src/e_dst accumulators (cols 0:8 src, 8:16 dst)
            eacc = cst.tile([N, 2 * HEADS], F32, tag="eacc")

            stage3 = sbt.tile([N, W3], F32, tag="stage3")
            cc_in = [
                dram.tile(
                    [N, GW if g < G - 1 else W3], F32,
                    tag=f"ccin{g}", name=f"ccin{g}",
                )
                for g in range(G)
            ]
            cc_out = [
                dram.tile(
                    [N, GW if g < G - 1 else W3], F32,
                    tag=f"ccout{g}", name=f"ccout{g}",
                )
                for g in range(G)
            ]
            g_bf = sbg.tile([N, HID], F16, tag="gbf")
            g3f = sbt.tile([N, W3], F32, tag="g3f")

            # ---- phase B: fp8 GEMM in G column groups, pipelined AllReduce ----
            with tc.tile_pool(name="psG", bufs=2, space="PSUM") as psG:
                ch = 0
                for g in range(G):
                    gps = psG.tile([N, GW], F32, tag="gps")
                    for c2 in range(NCH):
                        w1c = sbw1.tile([128, TPD, GW], F8, tag="w1")
                        off = (g * KT + c2 * TPD) * GW
                        q = nc.sync if ch % 2 == 0 else nc.scalar
                        q.dma_start(w1c[:], w1.ap()[:, off:off + TPD * GW])
                        ch += 1
                        for t in range(TPD):
                            k = c2 * TPD + t
                            nc.tensor.matmul(
                                gps[:],
                                xT[:, k, :],
                                w1c[:, t, :],
                                start=(k == 0),
                                stop=(k == KT - 1),
                            )
                    # descale partial g out of PSUM
                    if g < G - 1:
                        stage = sbst.tile([N, GW], F32, tag="st")
                        st_ap = stage[:]
                    else:
                        stage = stage3
                        st_ap = stage3[:, 0:GW]
                    nc.vector.tensor_scalar_mul(st_ap, gps[:], DESCALE)

                    # partial e_src/e_dst for this group's 2 heads
                    tmp = sbst.tile([N, GW], F32, tag="etmp")
                    nc.vector.tensor_mul(
                        tmp[:], st_ap, asrcb[:, GW * g:GW * (g + 1)]
                    )
                    nc.vector.tensor_reduce(
                        eacc[:, HPG * g:HPG * (g + 1)],
                        tmp[:].rearrange("p (h f) -> p h f", h=HPG),
                        axis=AX.X,
                        op=OP.add,
                    )
                    nc.vector.tensor_mul(
                        tmp[:], st_ap, adstb[:, GW * g:GW * (g + 1)]
                    )
                    nc.vector.tensor_reduce(
                        eacc[:, HEADS + HPG * g:HEADS + HPG * (g + 1)],
                        tmp[:].rearrange("p (h f) -> p h f", h=HPG),
                        axis=AX.X,
                        op=OP.add,
                    )
                    if g == G - 1:
                        nc.vector.tensor_copy(
                            stage3[:, GW:W3], eacc[:]
                        )
                    # ship partial to the collective
                    nc.sync.dma_start(cc_in[g][:], stage[:])
                    nc.gpsimd.collective_compute(
                        "AllReduce",
                        OP.add,
                        replica_groups=[list(range(NCORES))],
                        ins=[cc_in[g][:].opt()],
                        outs=[cc_out[g][:].opt()],
                    )
                    if g < G - 1:
                        gf = sbgf.tile([N, GW], F32, tag="gf")
                        nc.sync.dma_start(gf[:], cc_out[g][:])
                        nc.vector.tensor_copy(
                            g_bf[:, GW * g:GW * (g + 1)], gf[:]
                        )
                    else:
                        nc.sync.dma_start(g3f[:], cc_out[g][:])
                        nc.vector.tensor_copy(
                            g_bf[:, GW * g:GW * (g + 1)], g3f[:, 0:GW]
                        )

            esrc = g3f[:, GW:GW + HEADS]           # [46, 8] fp32
            edst = g3f[:, GW + HEADS:W3]           # [46, 8] fp32

            with (
                tc.tile_pool(name="psE", bufs=1, space="PSUM") as psE,
                tc.tile_pool(name="psH", bufs=1, space="PSUM") as psH,
                tc.tile_pool(name="psS", bufs=1, space="PSUM") as psS,
            ):
                # ---- phase C: layer-1 attention (8 heads) ----
                # edst[j,h] -> edstT[h,j] -> broadcast over i via sel matmuls
                edT_ps = psT.tile([HEADS, N], F32, tag="tp")
                nc.tensor.transpose(edT_ps[:], edst, ident_sb[:])
                edT = sbsm.tile([HEADS, N], F32, tag="edT")
                nc.vector.tensor_copy(edT[:], edT_ps[:])
                ebc_ps = psE.tile([N, HEADS, N], F32, tag="ebc")
                for h in range(HEADS):
                    nc.tensor.matmul(
                        ebc_ps[:, h, :],
                        sel_sb[:, N * h:N * (h + 1)],
                        edT[:],
                        start=True,
                        stop=True,
                    )
                # e = leaky_relu(esrc_i + edst_j, 0.2); u = exp(e) * adj
                e_sb = sbsm.tile([N, HEADS, N], F32, tag="e")
                nc.vector.tensor_add(
                    e_sb[:],
                    ebc_ps[:],
                    esrc.unsqueeze(2).broadcast_to([N, HEADS, N]),
                )
                t02 = sbsm.tile([N, HEADS, N], F32, tag="t02")
                nc.vector.tensor_scalar_mul(t02[:], e_sb[:], 0.2)
                nc.vector.tensor_max(e_sb[:], e_sb[:], t02[:])
                u_sb = sbsm.tile([N, HEADS, N], F32, tag="u")
                nc.scalar.activation(u_sb[:], e_sb[:], ACTF.Exp)
                nc.vector.tensor_mul(
                    u_sb[:],
                    u_sb[:],
                    adj01_sb[:].unsqueeze(1).broadcast_to([N, HEADS, N]),
                )
                s_sb = sbsm.tile([N, HEADS], F32, tag="s")
                nc.vector.tensor_reduce(s_sb[:], u_sb[:], axis=AX.X, op=OP.add)
                r_sb = sbsm.tile([N, HEADS], F32, tag="r")
                nc.vector.reciprocal(r_sb[:], s_sb[:])
                # fold softmax denominator into u before the matmul
                nc.vector.tensor_mul(
                    u_sb[:],
                    u_sb[:],
                    r_sb[:].unsqueeze(2).broadcast_to([N, HEADS, N]),
                )

                # h1T[f,i] per 128-block, via uT (att^T) as moving operand
                uT = sbsm.tile([N, HEADS, N], F16, tag="uT")
                for h in range(HEADS):
                    uT_ps = psT.tile([N, N], F32, tag="tp")
                    nc.tensor.transpose(uT_ps[:], u_sb[:, h, :], ident_sb[:])
                    nc.vector.tensor_copy(uT[:, h, :], uT_ps[:])
                h1T_ps = psH.tile([128, KT2, OUTF], F32, tag="big")
                for h in range(HEADS):
                    for b in range(2):
                        blk = 2 * h + b
                        nc.tensor.matmul(
                            h1T_ps[:, blk, 0:N],
                            g_bf[:, F1 * h + 128 * b:F1 * h + 128 * (b + 1)],
                            uT[:, h, :],
                            start=True,
                            stop=True,
                        )
                # ELU on the transposed h1, emit fp16 for layer-2
                tneg = sbt.tile([128, KT2, N], F32, tag="tneg")
                nc.vector.tensor_scalar_min(tneg[:], h1T_ps[:, :, 0:N], 0.0)
                texp = sbt.tile([128, KT2, N], F32, tag="texp")
                nc.scalar.activation(texp[:], tneg[:], ACTF.Exp)
                tpos = sbt.tile([128, KT2, N], F32, tag="tpos")
                nc.vector.tensor_scalar_max(tpos[:], h1T_ps[:, :, 0:N], 0.0)
                h_sb = sbt.tile([128, KT2, N], F16, tag="h")
                nc.vector.scalar_tensor_tensor(
                    h_sb[:], texp[:], -1.0, tpos[:], op0=OP.add, op1=OP.add
                )

                # ---- phase D: layer-2 GEMM + 1-head attention + MLP ----
                g2_ps = psH.tile([N, OUTF], F32, tag="big")
                for k in range(KT2):
                    nc.tensor.matmul(
                        g2_ps[:],
                        h_sb[:, k, :],
                        w2_sb[:, k, :],
                        start=(k == 0),
                        stop=(k == KT2 - 1),
                    )
                g2_sb = sbsm.tile([N, OUTF], F32, tag="g2")
                nc.vector.tensor_copy(g2_sb[:], g2_ps[:])
                g2b = sbsm.tile([N, OUTF], F16, tag="g2b")
                nc.vector.tensor_copy(g2b[:], g2_ps[:])

                t2 = sbsm.tile([N, OUTF], F32, tag="t2")
                nc.vector.tensor_mul(t2[:], g2_sb[:], a2sb[:])
                e2s = sbsm.tile([N, 1], F32, tag="e2s")
                nc.vector.tensor_reduce(e2s[:], t2[:], axis=AX.X, op=OP.add)
                nc.vector.tensor_mul(t2[:], g2_sb[:], a2db[:])
                e2d = sbsm.tile([N, 1], F32, tag="e2d")
                nc.vector.tensor_reduce(e2d[:], t2[:], axis=AX.X, op=OP.add)

                e2dT_ps = psS.tile([1, N], F32, tag="er")
                nc.tensor.transpose(e2dT_ps[:], e2d[:], ident_sb[:])
                e2dT = sbsm.tile([1, N], F32, tag="e2dT")
                nc.vector.tensor_copy(e2dT[:], e2dT_ps[:])
                ebc2_ps = psS.tile([N, N], F32, tag="er")
                nc.tensor.matmul(
                    ebc2_ps[:], ones46_sb[:], e2dT[:], start=True, stop=True
                )
                e2_sb = sbsm.tile([N, N], F32, tag="e2")
                nc.vector.tensor_scalar(
                    e2_sb[:], ebc2_ps[:], e2s[:, 0:1], None, OP.add
                )
                t22 = sbsm.tile([N, N], F32, tag="t22")
                nc.vector.tensor_scalar_mul(t22[:], e2_sb[:], 0.2)
                nc.vector.tensor_max(e2_sb[:], e2_sb[:], t22[:])
                u2_sb = sbsm.tile([N, N], F32, tag="u2")
                nc.scalar.activation(u2_sb[:], e2_sb[:], ACTF.Exp)
                nc.vector.tensor_mul(u2_sb[:], u2_sb[:], adj01_sb[:])
                s2_sb = sbsm.tile([N, 1], F32, tag="s2")
                nc.vector.tensor_reduce(s2_sb[:], u2_sb[:], axis=AX.X, op=OP.add)
                r2_sb = sbsm.tile([N, 1], F32, tag="r2")
                nc.vector.reciprocal(r2_sb[:], s2_sb[:])
                nc.vector.tensor_scalar(
                    u2_sb[:], u2_sb[:], r2_sb[:, 0:1], None, OP.mult
                )
                u2T_ps = psT.tile([N, N], F32, tag="tp")
                nc.tensor.transpose(u2T_ps[:], u2_sb[:], ident_sb[:])
                u2T = sbsm.tile([N, N], F16, tag="u2T")
                nc.vector.tensor_copy(u2T[:], u2T_ps[:])
                o2_ps = psH.tile([N, OUTF], F32, tag="big")
                nc.tensor.matmul(o2_ps[:], u2T[:], g2b[:], start=True, stop=True)
                # mean over the 64 features folded into host-prescaled mw1 (/64)
                m_sb = sbsm.tile([N, 1], F32, tag="m")
                nc.vector.tensor_reduce(m_sb[:], o2_ps[:], axis=AX.X, op=OP.add)

                z1_ps = psS.tile([1, 12], F32, tag="er")
                nc.tensor.matmul(z1_ps[:], m_sb[:], mw1_sb[:], start=True, stop=True)
                z1_sb = sbsm.tile([1, 12], F32, tag="z1")
                nc.vector.tensor_add(z1_sb[:], z1_ps[:], mb1_sb[:])
                zt_sb = sbsm.tile([1, 12], F32, tag="zt")
                nc.vector.tensor_mul(zt_sb[:], z1_sb[:], mw2t_sb[:])
                z2_sb = sbsm.tile([1, 1], F32, tag="z2")
                nc.vector.tensor_reduce(z2_sb[:], zt_sb[:], axis=AX.X, op=OP.add)
                # sigmoid(z2 + mb2) via exp (avoids a Sigmoid table load)
                zb_sb = sbsm.tile([1, 1], F32, tag="zb")
                nc.vector.tensor_add(zb_sb[:], z2_sb[:], mb2_sb[:])
                zn_sb = sbsm.tile([1, 1], F32, tag="zn")
                nc.vector.tensor_scalar_mul(zn_sb[:], zb_sb[:], -1.0)
                ez_sb = sbsm.tile([1, 1], F32, tag="ez")
                nc.scalar.activation(ez_sb[:], zn_sb[:], ACTF.Exp)
                ez1_sb = sbsm.tile([1, 1], F32, tag="ez1")
                nc.vector.tensor_scalar_add(ez1_sb[:], ez_sb[:], 1.0)
                res_sb = sbsm.tile([1, 1], F32, tag="res")
                nc.vector.reciprocal(res_sb[:], ez1_sb[:])
                nc.sync.dma_start(out.ap(), res_sb[:])

    nc.compile()
    return nc


_NC_CACHE = []


def _get_nc():
    if not _NC_CACHE:
        _NC_CACHE.append(build())
    return _NC_CACHE[0]


def _prep_in_maps(x, adj, W1, a1, W2, a2, mw1, mb1, mw2, mb2):
    adj01 = adj[:, :, 0].astype(np.float32)
    shared = {
        "adj01": adj01,
        "asrcf": np.ascontiguousarray(
            a1[:, :F1].reshape(1, HID).astype(np.float16)
        ),
        "adstf": np.ascontiguousarray(
            a1[:, F1:].reshape(1, HID).astype(np.float16)
        ),
        "a2sf": np.ascontiguousarray(a2[0, :OUTF].reshape(1, OUTF).astype(np.float16)),
        "a2df": np.ascontiguousarray(a2[0, OUTF:].reshape(1, OUTF).astype(np.float16)),
        "w2r": np.ascontiguousarray(
            W2.reshape(KT2, 128, OUTF).transpose(1, 0, 2).reshape(128, KT2 * OUTF)
        ).astype(np.float16),
        "sel": np.kron(np.eye(HEADS, dtype=np.float32), np.ones((1, N), np.float32)),
        "ones46": np.ones((1, N), np.float32),
        "mw1": np.ascontiguousarray(mw1 / np.float32(OUTF)),
        "mb1": mb1.reshape(1, 12).astype(np.float32),
        "mw2t": np.ascontiguousarray(mw2.reshape(1, 12)),
        "mb2": mb2.reshape(1, 1).astype(np.float32),
        "ident": np.eye(N, dtype=np.float32),
    }
    x8 = (x * np.float32(X_SCALE)).astype(NP_F8)
    W18 = (W1 * np.float32(W_SCALE)).astype(NP_F8)
    in_maps = []
    for c in range(NCORES):
        m = dict(shared)
        # x slice, transposed to [128, kt, 46]
        xc = np.ascontiguousarray(x8[:, KC * c:KC * (c + 1)]).T  # [16384, 46]
        m["xs"] = np.ascontiguousarray(
            xc.reshape(KT, 128, N).transpose(1, 0, 2).reshape(128, KT * N)
        )
        # W1 slice, [128, G, KT, GW] flattened
        w1c = W18[KC * c:KC * (c + 1), :].reshape(KT, 128, G, GW)
        m["w1"] = np.ascontiguousarray(
            w1c.transpose(1, 2, 0, 3).reshape(128, G * KT * GW)
        )
        in_maps.append(m)
    return in_maps


def kernel(**inputs):
    x = np.asarray(inputs["x"], dtype=np.float32)
    adj = np.asarray(inputs["adj_mat"]).astype(bool).reshape(N, N, 1)
    W1 = np.asarray(inputs["W1"], dtype=np.float32)
    a1 = np.asarray(inputs["a1"], dtype=np.float32)
    W2 = np.asarray(inputs["W2"], dtype=np.float32)
    a2 = np.asarray(inputs["a2"], dtype=np.float32)
    mw1 = np.asarray(inputs["mlp_w1"], dtype=np.float32)
    mb1 = np.asarray(inputs["mlp_b1"], dtype=np.float32)
    mw2 = np.asarray(inputs["mlp_w2"], dtype=np.float32)
    mb2 = np.asarray(inputs["mlp_b2"], dtype=np.float32)

    nc = _get_nc()
    in_maps = _prep_in_maps(x, adj, W1, a1, W2, a2, mw1, mb1, mw2, mb2)
    res = run_bass_kernel_spmd(nc, in_maps, core_ids=list(range(NCORES)))
    return res.results[0]["out"].reshape(1).astype(np.float32)
